# revision 29
# baseline (speedup 1.0000x reference)
"""Self-contained Trainium2 Bass kernel for nn_Attention_40226663694923.

Fused LayerNorm + multi-head attention + out-projection, sharded over
8 NeuronCores as (batch b in 0..3) x (head-group g in 0..1, 8 heads each).
Each core receives x[b].T plus its weight shards, computes a partial
out-projection [2048, 1024]; the host sums the two head-group partials
per batch and adds the bias.

Device-side layout is fully "transposed" (feature dim on partitions):
  - LN stats via ones-matmuls on PE (cross-partition sums), Rsqrt on ACT,
    rank-1 PE broadcast of rstd / mu*rstd rows copied to SBUF, in-place
    normalize of x^T (stats for all token blocks first, then normalize,
    so the DVE chain overlaps phase B's projection matmuls).
  - q/k projections packed two heads per matmul (M=128), v projection
    in natural layout for all 8 heads at once (N=512).
  - logitsT[tk, tq] = k^T.T-slices @ q into a persistent 4-bank PSUM
    tile (half-bank ping-pong): exp runs as two [128,1024] ACT calls per
    ck so next ck's logits overlap the current exp; attn@V via
    lhsT=[V|1] (ones column yields softmax denominators for free).
  - Tail per pair: reciprocal_approx_fast on the denominator row, PE
    rank-1 broadcast, ACT copy to SBUF, DVE multiply (no DRAM bounce).
  - out-projection from packed per-pair O tiles, full PSUM accumulation.
All matmuls run in float32r (full-rate fp32 on the PE at N>=256).
"""

import os
import sys

for _p in ("/opt/trn_rl_repo", "/root/.axon_site/_ro/trn_rl_repo"):
    if os.path.isdir(_p) and _p not in sys.path:
        sys.path.append(_p)

from contextlib import ExitStack

import numpy as np

B, N, DIM = 4, 2048, 1024
H, D = 16, 64
HPC = 8        # heads per core
PAIRS = 4      # head pairs per core
KC = 8         # 1024 dim contraction chunks of 128
NB = 4         # token blocks of 512
TB = 512
TT = 16        # token tiles of 128
LN_EPS = 1e-6
N_CORES = 8

_prog_cache = {}


def _build_program():
    import concourse.bass as bass
    import concourse.mybir as mybir
    import concourse.tile as tile
    from concourse import bacc

    F32 = mybir.dt.float32
    F32R = mybir.dt.float32r
    BF16 = mybir.dt.bfloat16
    AF = mybir.ActivationFunctionType

    nc = bacc.Bacc("TRN2", target_bir_lowering=False, debug=False)
    xt_d = nc.dram_tensor("xt", [DIM, N], F32R, kind="ExternalInput")
    wq_d = nc.dram_tensor("wq", [PAIRS, 128, KC, 128], F32R, kind="ExternalInput")
    wk_d = nc.dram_tensor("wk", [128, KC, PAIRS, 128], F32R, kind="ExternalInput")
    wv_d = nc.dram_tensor("wv", [128, KC, 512], F32R, kind="ExternalInput")
    wo_d = nc.dram_tensor("wo", [128, PAIRS, 1024], F32R, kind="ExternalInput")
    qb_d = nc.dram_tensor("qb", [PAIRS, 128], F32, kind="ExternalInput")
    kb_d = nc.dram_tensor("kb", [PAIRS, 128], F32, kind="ExternalInput")
    out_d = nc.dram_tensor("out", [N, DIM], F32, kind="ExternalOutput")

    with tile.TileContext(nc) as tc, ExitStack() as ctx:
        const_p = ctx.enter_context(tc.tile_pool(name="const", bufs=1))
        big_p = ctx.enter_context(tc.tile_pool(name="big", bufs=1))

        onesF = const_p.tile([128, 128], F32)
        nc.vector.memset(onesF, 1.0)
        ones_col = const_p.tile([128, 1], F32R)
        nc.vector.tensor_copy(out=ones_col, in_=onesF[:, 0:1])
        ones_row = const_p.tile([1, 128], F32R)
        nc.vector.tensor_copy(out=ones_row, in_=onesF[0:1, :])
        ones_row_bf = const_p.tile([1, 128], BF16)
        nc.vector.tensor_copy(out=ones_row_bf, in_=onesF[0:1, :])
        eps1 = const_p.tile([1, 1], F32)
        nc.vector.memset(eps1, LN_EPS)
        zb128 = const_p.tile([128, 1], F32)
        nc.vector.memset(zb128, 0.0)
        qb_sb = const_p.tile([128, PAIRS], F32)
        kb_sb = const_p.tile([128, PAIRS], F32)
        for pair in range(PAIRS):
            nc.gpsimd.dma_start(
                out=qb_sb[:, pair : pair + 1],
                in_=qb_d[pair, :].rearrange("(p one) -> p one", one=1),
            )
            nc.gpsimd.dma_start(
                out=kb_sb[:, pair : pair + 1],
                in_=kb_d[pair, :].rearrange("(p one) -> p one", one=1),
            )

        # persistent big tensors
        xt_sb = big_p.tile([128, KC, N], F32R)      # becomes z (normalized) in place
        k_sb = big_p.tile([128, PAIRS, N], BF16)    # kT, two heads packed per pair
        v_sb = big_p.tile([128, TT, HPC, D + 1], BF16)  # V natural + ones column
        wo_sb = big_p.tile([128, PAIRS, 1024], F32R)

        # xt: per-(kc, tb) pieces so phase A's stats for tb=0 can start after
        # only 8 small DMAs (tb-major issue order). sync queue.
        for tb in range(NB):
            for kc in range(KC):
                nc.sync.dma_start(
                    out=xt_sb[:, kc, tb * TB : (tb + 1) * TB],
                    in_=xt_d[kc * 128 : (kc + 1) * 128, tb * TB : (tb + 1) * TB],
                )
        nc.sync.dma_start(out=wo_sb, in_=wo_d[:, :, :])
        nc.vector.tensor_copy(
            out=v_sb[:, :, :, D : D + 1],
            in_=onesF.rearrange("p (a b c) -> p a b c", a=TT, b=HPC),
        )
        # k/v weights on the gpsimd queue so they don't sit behind xt; the
        # wkv pool closes after phase B so its SBUF is reused by phase C.
        wkv_ctx = tc.tile_pool(name="wkv", bufs=1)
        wkv_p = wkv_ctx.__enter__()
        wv_sb = wkv_p.tile([128, KC, 512], F32R, tag="wv")
        nc.gpsimd.dma_start(out=wv_sb, in_=wv_d[:, :, :])
        wk_sb = wkv_p.tile([128, KC, PAIRS, 128], F32R, tag="wk")
        nc.gpsimd.dma_start(out=wk_sb, in_=wk_d[:, :, :, :])

        # ------- Phase A+B merged: per-tb stats -> normalize -> k/v proj ------
        # One per-tb pipeline so the PE runs tb's projections while the DVE
        # chews tb+1's stats/normalize chain. All accumulators live in one
        # 8-bank pool that closes before phase C's po/pl pool opens.
        with tc.tile_pool(name="psA", bufs=1, space="PSUM") as psA, \
             tc.tile_pool(name="sqp", bufs=3) as sqp, \
             tc.tile_pool(name="rows", bufs=1) as rows:
            for tb in range(NB):
                ts_ = slice(tb * TB, (tb + 1) * TB)
                s1 = psA.tile([1, TB], F32, tag="s1", bufs=1)
                s2 = psA.tile([1, TB], F32, tag="s2", bufs=1)
                for kc in range(KC):
                    sq = sqp.tile([128, TB], F32R, tag="sq")
                    nc.scalar.activation(out=sq, in_=xt_sb[:, kc, ts_].bitcast(F32),
                                         func=AF.Square, bias=zb128[:, 0:1])
                    nc.tensor.matmul(s1, lhsT=ones_col, rhs=xt_sb[:, kc, ts_],
                                     start=(kc == 0), stop=(kc == KC - 1))
                    nc.tensor.matmul(s2, lhsT=ones_col, rhs=sq,
                                     start=(kc == 0), stop=(kc == KC - 1))
                mu = rows.tile([1, TB], F32, tag="mu")
                nc.vector.tensor_scalar_mul(mu, s1, 1.0 / DIM)
                ex2 = rows.tile([1, TB], F32, tag="ex2")
                nc.vector.tensor_scalar_mul(ex2, s2, 1.0 / DIM)
                var_r = rows.tile([1, TB], F32, tag="var")
                nc.vector.tensor_mul(var_r, mu, mu)
                nc.vector.tensor_sub(var_r, ex2, var_r)
                sd = rows.tile([1, TB], F32, tag="sd")
                nc.scalar.activation(out=sd, in_=var_r, func=AF.Sqrt,
                                     bias=eps1[0:1, 0:1])
                rstd_r = rows.tile([1, TB], F32, tag="rstd_r")
                nc.vector.reciprocal(out=rstd_r, in_=sd)
                murstd_r = rows.tile([1, TB], F32R, tag="murstd")
                nc.vector.tensor_mul(murstd_r, mu, rstd_r)
                rstd_rr = rows.tile([1, TB], F32R, tag="rstd_rr")
                nc.vector.tensor_copy(out=rstd_rr, in_=rstd_r)
                rb1 = psA.tile([128, TB], F32, tag="rb1", bufs=1)
                nc.tensor.matmul(rb1, lhsT=ones_row, rhs=rstd_rr,
                                 start=True, stop=True)
                rb2 = psA.tile([128, TB], F32, tag="rb2", bufs=1)
                nc.tensor.matmul(rb2, lhsT=ones_row, rhs=murstd_r,
                                 start=True, stop=True)
                for kc in range(KC):
                    nc.vector.tensor_mul(xt_sb[:, kc, ts_],
                                         xt_sb[:, kc, ts_].bitcast(F32), rb1)
                    nc.vector.tensor_sub(xt_sb[:, kc, ts_],
                                         xt_sb[:, kc, ts_].bitcast(F32), rb2)
                for pair in range(PAIRS):
                    pk = psA.tile([128, TB], F32, tag="acc", bufs=4)
                    for kc in range(KC):
                        nc.tensor.matmul(pk, lhsT=wk_sb[:, kc, pair, :],
                                         rhs=xt_sb[:, kc, ts_],
                                         start=(kc == 0), stop=(kc == KC - 1))
                    nc.vector.tensor_scalar_add(out=k_sb[:, pair, ts_], in0=pk,
                                                scalar1=kb_sb[:, pair : pair + 1])
                for tt in range(tb * 4, tb * 4 + 4):
                    tts = slice(tt * 128, (tt + 1) * 128)
                    pv = psA.tile([128, 512], F32, tag="acc", bufs=4)
                    for kc in range(KC):
                        nc.tensor.matmul(pv, lhsT=xt_sb[:, kc, tts],
                                         rhs=wv_sb[:, kc, :],
                                         start=(kc == 0), stop=(kc == KC - 1))
                    nc.vector.tensor_copy(
                        out=v_sb[:, tt, :, 0:D],
                        in_=pv.rearrange("p (h d) -> p h d", h=HPC),
                    )
        wkv_ctx.__exit__(None, None, None)

        psB = ctx.enter_context(tc.tile_pool(name="psB", bufs=2, space="PSUM"))

        # ---------------- Phase C: attention + out-projection -----------------
        with tc.tile_pool(name="attn", bufs=2) as ap_, \
             tc.tile_pool(name="rows2", bufs=2) as rows2, \
             tc.tile_pool(name="drbounce", bufs=4, space="DRAM") as dram_p:
            for tqb in range(NB):
                tqs_ = slice(tqb * TB, (tqb + 1) * TB)
                obuf = ap_.tile([128, PAIRS, TB], F32R, tag="ob", bufs=1)
                # --- all 4 q projections upfront, into one pl-tag slot ------
                pq4 = psB.tile([128, 2048], F32, tag="pl", bufs=1)
                q_sb = ap_.tile([128, PAIRS, TB], BF16, tag="q", bufs=1)
                for pair in range(PAIRS):
                    wq_sb = ap_.tile([128, KC, 128], F32R, tag="wq")
                    nc.sync.dma_start(out=wq_sb, in_=wq_d[pair])
                    for kc in range(KC):
                        nc.tensor.matmul(pq4[:, pair * TB : (pair + 1) * TB],
                                         lhsT=wq_sb[:, kc, :],
                                         rhs=xt_sb[:, kc, tqs_],
                                         start=(kc == 0), stop=(kc == KC - 1))
                    nc.vector.tensor_scalar_add(
                        out=q_sb[:, pair, :],
                        in0=pq4[:, pair * TB : (pair + 1) * TB],
                        scalar1=qb_sb[:, pair : pair + 1])
                def emit_tail(tpair, tpo2):
                    # Softmax-normalize tail with NO PE work: reciprocal of
                    # the ones-row on DVE, row-broadcast via a DRAM-bounce
                    # DMA pair (gpsimd queue is otherwise idle), DVE multiply.
                    import concourse.bass as _b
                    for hh in range(2):
                        po = tpo2[hh]
                        rrow = rows2.tile([1, TB], F32, tag="rr")
                        nc.vector.reciprocal(out=rrow, in_=po[D : D + 1, :])
                        dr = dram_p.tile([1, TB], F32, tag="dr")
                        nc.gpsimd.dma_start(out=dr, in_=rrow)
                        rb = ap_.tile([64, TB], F32, tag="rb", bufs=2)
                        bc = _b.AP(tensor=dr.tensor, offset=dr.offset,
                                   ap=[[0, 64]] + [list(p) for p in dr[0, :].ap])
                        nc.gpsimd.dma_start(out=rb, in_=bc)
                        if hh == 0:
                            nc.vector.tensor_mul(obuf[0:64, tpair, :],
                                                 po[0:D, :], rb)
                        else:
                            tmp = ap_.tile([64, TB], F32R, tag="tmp")
                            nc.vector.tensor_mul(tmp, po[0:D, :], rb)
                            nc.gpsimd.dma_start(out=obuf[64:128, tpair, :],
                                                in_=tmp)

                def make_exp_v(epair, etkc, epl, epo2):
                    # exp shifted one step late, attn@V two steps late: no
                    # engine ever waits on a dependency issued the same step.
                    half = (etkc % 2) * 1024
                    cell = {}

                    def emit_exp():
                        ex = ap_.tile([128, 1024], BF16, tag="ex", bufs=4)
                        nc.scalar.activation(
                            out=ex, in_=epl[:, half : half + 1024],
                            func=AF.Exp, bias=zb128[:, 0:1])
                        cell["ex"] = ex

                    def emit_v():
                        ex = cell["ex"]
                        for hh in range(2):
                            nc.tensor.matmul(
                                epo2[hh][0 : D + 1, :],
                                lhsT=v_sb[:, etkc, epair * 2 + hh, :],
                                rhs=ex[:, hh * 512 : (hh + 1) * 512],
                                start=(etkc == 0), stop=(etkc == 2 * KC - 1))
                    return emit_exp, emit_v

                from collections import deque
                expq = deque()
                vq = deque()
                pending_t = None
                for pair in range(PAIRS):
                    po0 = psB.tile([128, TB], F32, tag="po", bufs=4)
                    po1 = psB.tile([128, TB], F32, tag="po", bufs=4)
                    po2 = [po0, po1]
                    # persistent 4-bank logits tile for this pair; 2-bank
                    # halves ping-pong by k-tile parity.
                    pl = psB.tile([128, 2048], F32, tag="pl", bufs=1)
                    for tkc in range(2 * KC):
                        for hh in range(2):
                            pb = hh * 64
                            nc.tensor.matmul(
                                pl[:, (tkc % 2) * 1024 + hh * 512 :
                                   (tkc % 2) * 1024 + (hh + 1) * 512],
                                lhsT=k_sb[pb : pb + 64, pair,
                                          tkc * 128 : (tkc + 1) * 128],
                                rhs=q_sb[pb : pb + 64, pair, :],
                                start=True, stop=True)
                        if expq:
                            expq.popleft()()
                        if len(vq) >= 3:
                            vq.popleft()()
                        if tkc == 3 and pending_t is not None:
                            emit_tail(*pending_t)
                            pending_t = None
                        e, v = make_exp_v(pair, tkc, pl, po2)
                        expq.append(e)
                        vq.append(v)
                    pending_t = (pair, po2)
                while expq:
                    expq.popleft()()
                    while len(vq) > len(expq) + 2:
                        vq.popleft()()
                while vq:
                    vq.popleft()()
                emit_tail(*pending_t)
                pending_t = None
                # out-projection for this tq block
                for tqs in range(4):
                    osl = slice(tqs * 128, (tqs + 1) * 128)
                    osb = ap_.tile([128, 1024], F32, tag="osb", bufs=2)
                    pc = psB.tile([128, 1024], F32, tag="pl", bufs=1)
                    for nh in range(2):
                        for j in range(PAIRS):
                            nc.tensor.matmul(
                                pc[:, nh * 512 : (nh + 1) * 512],
                                lhsT=obuf[:, j, osl],
                                rhs=wo_sb[:, j, nh * 512 : (nh + 1) * 512],
                                start=(j == 0), stop=(j == PAIRS - 1))
                        if nh == 0:
                            nc.scalar.copy(out=osb[:, 0:512],
                                           in_=pc[:, 0:512])
                        else:
                            nc.vector.tensor_copy(out=osb[:, 512:1024],
                                                  in_=pc[:, 512:1024])
                    r0 = tqb * TB + tqs * 128
                    nc.sync.dma_start(out=out_d[r0 : r0 + 128, :], in_=osb)

    nc.finalize()
    return nc


def get_program():
    if "nc" not in _prog_cache:
        _prog_cache["nc"] = _build_program()
    return _prog_cache["nc"]


def _round_f32r(a):
    """Round fp32 to fp32r (E8M11: 11 mantissa bits, low 12 bits zero),
    round-to-nearest-even. Matches the PE's fp32r operand precision so the
    DMA-loaded tensors satisfy walrus's 'rounded to FP32r' requirement."""
    b = np.ascontiguousarray(a, np.float32).view(np.uint32)
    lsb = (b >> np.uint32(12)) & np.uint32(1)
    r = (b + np.uint32(0x7FF) + lsb) & np.uint32(0xFFFFF000)
    return r.view(np.float32)


def _pack_inputs(x, ln_scale, ln_bias, w_qkv, w_out, b_out):
    """Returns (in_maps for 8 cores, per-batch host bias [1024])."""
    x = np.ascontiguousarray(np.asarray(x, np.float32))
    ln_scale = np.asarray(ln_scale, np.float32)
    ln_bias = np.asarray(ln_bias, np.float32)
    w_qkv = np.asarray(w_qkv, np.float32)
    w_out = np.asarray(w_out, np.float32)
    b_out = np.asarray(b_out, np.float32)

    ws = w_qkv * ln_scale[:, None]          # fold LN scale into weights
    wq_all = ws[:, 0:1024] * (D ** -0.5)    # fold 1/sqrt(d) into q
    wk_all = ws[:, 1024:2048]
    wv_all = ws[:, 2048:3072]
    qb_all = (ln_bias @ w_qkv[:, 0:1024]) * (D ** -0.5)
    kb_all = ln_bias @ w_qkv[:, 1024:2048]
    vb_all = ln_bias @ w_qkv[:, 2048:3072]
    b_eff = (b_out + vb_all @ w_out).astype(np.float32)  # host-side bias

    in_maps = []
    for core in range(N_CORES):
        b_i, g = core // 2, core % 2
        cs = slice(g * 512, (g + 1) * 512)
        # [dim, 8 heads, 64] -> pairs of heads packed along m
        wq_g = wq_all[:, cs].reshape(DIM, PAIRS, 128)   # [dim, pair, 2*64]
        wk_g = wk_all[:, cs].reshape(DIM, PAIRS, 128)
        # -> [pair, p, kc, m] so that per-pair DMA is contiguous per partition
        wq_p = np.ascontiguousarray(
            wq_g.reshape(KC, 128, PAIRS, 128).transpose(2, 1, 0, 3))
        wk_p = np.ascontiguousarray(
            wk_g.reshape(KC, 128, PAIRS, 128).transpose(1, 0, 2, 3))
        wv_p = np.ascontiguousarray(
            wv_all[:, cs].reshape(KC, 128, 512).transpose(1, 0, 2))
        wo_p = np.ascontiguousarray(
            w_out[cs, :].reshape(PAIRS, 128, DIM).transpose(1, 0, 2))
        qb_p = np.ascontiguousarray(qb_all[cs].reshape(PAIRS, 128))
        kb_p = np.ascontiguousarray(kb_all[cs].reshape(PAIRS, 128))
        xt = np.ascontiguousarray(x[b_i].T)
        in_maps.append({
            "xt": _round_f32r(xt), "wq": _round_f32r(wq_p),
            "wk": _round_f32r(wk_p), "wv": _round_f32r(wv_p),
            "wo": _round_f32r(wo_p), "qb": qb_p, "kb": kb_p,
        })
    return in_maps, b_eff


def kernel(x, ln_scale, ln_bias, w_qkv, w_out, b_out):
    from concourse.bass_utils import run_bass_kernel_spmd

    nc = get_program()
    in_maps, b_eff = _pack_inputs(x, ln_scale, ln_bias, w_qkv, w_out, b_out)
    trace = bool(os.environ.get("ATTN_KERNEL_TRACE"))
    res = run_bass_kernel_spmd(nc, in_maps, core_ids=list(range(N_CORES)),
                               trace=trace)
    _prog_cache["last_exec_time_ns"] = res.exec_time_ns
    _prog_cache["last_result"] = res
    outs = res.results
    out = np.empty((B, N, DIM), np.float32)
    for b in range(B):
        out[b] = outs[2 * b]["out"] + outs[2 * b + 1]["out"] + b_eff
    return out


# revision 30
# speedup vs baseline: 1.1517x; 1.1517x over previous
"""Self-contained Trainium2 Bass kernel for nn_Attention_40226663694923.

Fused LayerNorm + multi-head attention + out-projection, sharded over
8 NeuronCores as (batch b in 0..3) x (head-group g in 0..1, 8 heads each).
Each core receives x[b].T plus its weight shards, computes a partial
out-projection [2048, 1024]; the host sums the two head-group partials
per batch and adds the bias.

Device-side layout is fully "transposed" (feature dim on partitions):
  - LN stats via ones-matmuls on PE (cross-partition sums), Rsqrt on ACT,
    rank-1 PE broadcast of rstd / mu*rstd rows copied to SBUF, in-place
    normalize of x^T (stats for all token blocks first, then normalize,
    so the DVE chain overlaps phase B's projection matmuls).
  - q/k projections packed two heads per matmul (M=128), v projection
    in natural layout for all 8 heads at once (N=512).
  - logitsT[tk, tq] = k^T.T-slices @ q into a persistent 4-bank PSUM
    tile (half-bank ping-pong): exp runs as two [128,1024] ACT calls per
    ck so next ck's logits overlap the current exp; attn@V via
    lhsT=[V|1] (ones column yields softmax denominators for free).
  - Tail per pair: reciprocal_approx_fast on the denominator row, PE
    rank-1 broadcast, ACT copy to SBUF, DVE multiply (no DRAM bounce).
  - out-projection from packed per-pair O tiles, full PSUM accumulation.
All matmuls run in float32r (full-rate fp32 on the PE at N>=256).
"""

import os
import sys

for _p in ("/opt/trn_rl_repo", "/root/.axon_site/_ro/trn_rl_repo"):
    if os.path.isdir(_p) and _p not in sys.path:
        sys.path.append(_p)

from contextlib import ExitStack

import numpy as np

B, N, DIM = 4, 2048, 1024
H, D = 16, 64
HPC = 8        # heads per core
PAIRS = 4      # head pairs per core
KC = 8         # 1024 dim contraction chunks of 128
NB = 4         # token blocks of 512
TB = 512
TT = 16        # token tiles of 128
LN_EPS = 1e-6
N_CORES = 8

_prog_cache = {}


def _build_program():
    import concourse.bass as bass
    import concourse.mybir as mybir
    import concourse.tile as tile
    from concourse import bacc

    F32 = mybir.dt.float32
    F32R = mybir.dt.float32r
    BF16 = mybir.dt.bfloat16
    AF = mybir.ActivationFunctionType

    nc = bacc.Bacc("TRN2", target_bir_lowering=False, debug=False)
    xt_d = nc.dram_tensor("xt", [DIM, N], F32R, kind="ExternalInput")
    wq_d = nc.dram_tensor("wq", [PAIRS, 128, KC, 128], F32R, kind="ExternalInput")
    wk_d = nc.dram_tensor("wk", [128, KC, PAIRS, 128], F32R, kind="ExternalInput")
    wv_d = nc.dram_tensor("wv", [128, KC, 512], F32R, kind="ExternalInput")
    wo_d = nc.dram_tensor("wo", [128, PAIRS, 1024], F32R, kind="ExternalInput")
    qb_d = nc.dram_tensor("qb", [PAIRS, 128], F32, kind="ExternalInput")
    kb_d = nc.dram_tensor("kb", [PAIRS, 128], F32, kind="ExternalInput")
    out_d = nc.dram_tensor("out", [N, DIM], F32, kind="ExternalOutput")

    with tile.TileContext(nc) as tc, ExitStack() as ctx:
        const_p = ctx.enter_context(tc.tile_pool(name="const", bufs=1))
        big_p = ctx.enter_context(tc.tile_pool(name="big", bufs=1))

        onesF = const_p.tile([128, 128], F32)
        nc.vector.memset(onesF, 1.0)
        ones_col = const_p.tile([128, 1], F32R)
        nc.vector.tensor_copy(out=ones_col, in_=onesF[:, 0:1])
        ones_row = const_p.tile([1, 128], F32R)
        nc.vector.tensor_copy(out=ones_row, in_=onesF[0:1, :])
        ones_row_bf = const_p.tile([1, 128], BF16)
        nc.vector.tensor_copy(out=ones_row_bf, in_=onesF[0:1, :])
        eps1 = const_p.tile([1, 1], F32)
        nc.vector.memset(eps1, LN_EPS)
        zb128 = const_p.tile([128, 1], F32)
        nc.vector.memset(zb128, 0.0)
        qb_sb = const_p.tile([128, PAIRS], F32)
        kb_sb = const_p.tile([128, PAIRS], F32)
        for pair in range(PAIRS):
            nc.gpsimd.dma_start(
                out=qb_sb[:, pair : pair + 1],
                in_=qb_d[pair, :].rearrange("(p one) -> p one", one=1),
            )
            nc.gpsimd.dma_start(
                out=kb_sb[:, pair : pair + 1],
                in_=kb_d[pair, :].rearrange("(p one) -> p one", one=1),
            )

        # persistent big tensors
        xt_sb = big_p.tile([128, KC, N], F32R)      # becomes z (normalized) in place
        k_sb = big_p.tile([128, PAIRS, N], BF16)    # kT, two heads packed per pair
        v_sb = big_p.tile([128, TT, HPC, D + 1], BF16)  # V natural + ones column
        wo_sb = big_p.tile([128, PAIRS, 1024], F32R)

        # xt: per-(kc, tb) pieces so phase A's stats for tb=0 can start after
        # only 8 small DMAs (tb-major issue order). sync queue.
        for tb in range(NB):
            for kc in range(KC):
                nc.sync.dma_start(
                    out=xt_sb[:, kc, tb * TB : (tb + 1) * TB],
                    in_=xt_d[kc * 128 : (kc + 1) * 128, tb * TB : (tb + 1) * TB],
                )
        nc.sync.dma_start(out=wo_sb, in_=wo_d[:, :, :])
        nc.vector.tensor_copy(
            out=v_sb[:, :, :, D : D + 1],
            in_=onesF.rearrange("p (a b c) -> p a b c", a=TT, b=HPC),
        )
        # k/v weights on the gpsimd queue so they don't sit behind xt; the
        # wkv pool closes after phase B so its SBUF is reused by phase C.
        wkv_ctx = tc.tile_pool(name="wkv", bufs=1)
        wkv_p = wkv_ctx.__enter__()
        wv_sb = wkv_p.tile([128, KC, 512], F32R, tag="wv")
        nc.gpsimd.dma_start(out=wv_sb, in_=wv_d[:, :, :])
        wk_sb = wkv_p.tile([128, KC, PAIRS, 128], F32R, tag="wk")
        nc.gpsimd.dma_start(out=wk_sb, in_=wk_d[:, :, :, :])

        # ---------------- Phase A: LN stats then in-place normalize ----------
        with tc.tile_pool(name="psA", bufs=2, space="PSUM") as psA, \
             tc.tile_pool(name="sqp", bufs=3) as sqp, \
             tc.tile_pool(name="rows", bufs=1) as rows:
            rb_tiles = []
            for tb in range(NB):
                ts_ = slice(tb * TB, (tb + 1) * TB)
                s1 = psA.tile([1, TB], F32, tag="s1", bufs=1)
                s2 = psA.tile([1, TB], F32, tag="s2", bufs=1)
                for kc in range(KC):
                    sq = sqp.tile([128, TB], F32R, tag="sq")
                    nc.scalar.activation(out=sq, in_=xt_sb[:, kc, ts_].bitcast(F32),
                                         func=AF.Square, bias=zb128[:, 0:1])
                    nc.tensor.matmul(s1, lhsT=ones_col, rhs=xt_sb[:, kc, ts_],
                                     start=(kc == 0), stop=(kc == KC - 1))
                    nc.tensor.matmul(s2, lhsT=ones_col, rhs=sq,
                                     start=(kc == 0), stop=(kc == KC - 1))
                mu = rows.tile([1, TB], F32, tag="mu")
                nc.vector.tensor_scalar_mul(mu, s1, 1.0 / DIM)
                ex2 = rows.tile([1, TB], F32, tag="ex2")
                nc.vector.tensor_scalar_mul(ex2, s2, 1.0 / DIM)
                var_r = rows.tile([1, TB], F32, tag="var")
                nc.vector.tensor_mul(var_r, mu, mu)
                nc.vector.tensor_sub(var_r, ex2, var_r)
                sd = rows.tile([1, TB], F32, tag="sd")
                nc.scalar.activation(out=sd, in_=var_r, func=AF.Sqrt,
                                     bias=eps1[0:1, 0:1])
                rstd_r = rows.tile([1, TB], F32, tag="rstd_r")
                nc.vector.reciprocal(out=rstd_r, in_=sd)
                murstd_r = rows.tile([1, TB], F32R, tag="murstd")
                nc.vector.tensor_mul(murstd_r, mu, rstd_r)
                rstd_rr = rows.tile([1, TB], F32R, tag="rstd_rr")
                nc.vector.tensor_copy(out=rstd_rr, in_=rstd_r)
                rb1 = psA.tile([128, TB], F32, tag="rb1", bufs=3)
                nc.tensor.matmul(rb1, lhsT=ones_row, rhs=rstd_rr,
                                 start=True, stop=True)
                rb2 = psA.tile([128, TB], F32, tag="rb2", bufs=3)
                nc.tensor.matmul(rb2, lhsT=ones_row, rhs=murstd_r,
                                 start=True, stop=True)
                rb_tiles.append((rb1, rb2))
            for tb in range(NB):
                ts_ = slice(tb * TB, (tb + 1) * TB)
                rb1, rb2 = rb_tiles[tb]
                for kc in range(KC):
                    nc.vector.tensor_mul(xt_sb[:, kc, ts_],
                                         xt_sb[:, kc, ts_].bitcast(F32), rb1)
                    nc.vector.tensor_sub(xt_sb[:, kc, ts_],
                                         xt_sb[:, kc, ts_].bitcast(F32), rb2)

        # ---------------- Phase B: k and v projections ------------------------
        psB = ctx.enter_context(tc.tile_pool(name="psB", bufs=2, space="PSUM"))
        for tb in range(NB):
            ts_ = slice(tb * TB, (tb + 1) * TB)
            for pair in range(PAIRS):
                pk = psB.tile([128, TB], F32, tag="po", bufs=4)
                for kc in range(KC):
                    nc.tensor.matmul(pk, lhsT=wk_sb[:, kc, pair, :],
                                     rhs=xt_sb[:, kc, ts_],
                                     start=(kc == 0), stop=(kc == KC - 1))
                nc.vector.tensor_scalar_add(out=k_sb[:, pair, ts_], in0=pk,
                                            scalar1=kb_sb[:, pair : pair + 1])
            for tt in range(tb * 4, tb * 4 + 4):
                tts = slice(tt * 128, (tt + 1) * 128)
                pv = psB.tile([128, 512], F32, tag="po", bufs=4)
                for kc in range(KC):
                    nc.tensor.matmul(pv, lhsT=xt_sb[:, kc, tts],
                                     rhs=wv_sb[:, kc, :],
                                     start=(kc == 0), stop=(kc == KC - 1))
                nc.vector.tensor_copy(
                    out=v_sb[:, tt, :, 0:D],
                    in_=pv.rearrange("p (h d) -> p h d", h=HPC),
                )
        wkv_ctx.__exit__(None, None, None)

        # ---------------- Phase C: attention + out-projection -----------------
        with tc.tile_pool(name="attn", bufs=2) as ap_, \
             tc.tile_pool(name="rows2", bufs=2) as rows2, \
             tc.tile_pool(name="drbounce", bufs=4, space="DRAM") as dram_p:
            for tqb in range(NB):
                tqs_ = slice(tqb * TB, (tqb + 1) * TB)
                obuf = ap_.tile([128, PAIRS, TB], F32R, tag="ob", bufs=1)
                # --- all 4 q projections upfront, into one pl-tag slot ------
                pq4 = psB.tile([128, 2048], F32, tag="pl", bufs=1)
                q_sb = ap_.tile([128, PAIRS, TB], BF16, tag="q", bufs=1)
                for pair in range(PAIRS):
                    wq_sb = ap_.tile([128, KC, 128], F32R, tag="wq")
                    nc.sync.dma_start(out=wq_sb, in_=wq_d[pair])
                    for kc in range(KC):
                        nc.tensor.matmul(pq4[:, pair * TB : (pair + 1) * TB],
                                         lhsT=wq_sb[:, kc, :],
                                         rhs=xt_sb[:, kc, tqs_],
                                         start=(kc == 0), stop=(kc == KC - 1))
                    nc.vector.tensor_scalar_add(
                        out=q_sb[:, pair, :],
                        in0=pq4[:, pair * TB : (pair + 1) * TB],
                        scalar1=qb_sb[:, pair : pair + 1])
                def emit_tail(tpair, tpo2):
                    # Softmax-normalize tail with NO PE work: reciprocal of
                    # the ones-row on DVE, row-broadcast via a DRAM-bounce
                    # DMA pair (gpsimd queue is otherwise idle), DVE multiply.
                    import concourse.bass as _b
                    for hh in range(2):
                        po = tpo2[hh]
                        rrow = rows2.tile([1, TB], F32, tag="rr")
                        nc.vector.reciprocal(out=rrow, in_=po[D : D + 1, :])
                        dr = dram_p.tile([1, TB], F32, tag="dr")
                        nc.gpsimd.dma_start(out=dr, in_=rrow)
                        rb = ap_.tile([64, TB], F32, tag="rb", bufs=2)
                        bc = _b.AP(tensor=dr.tensor, offset=dr.offset,
                                   ap=[[0, 64]] + [list(p) for p in dr[0, :].ap])
                        nc.gpsimd.dma_start(out=rb, in_=bc)
                        if hh == 0:
                            nc.vector.tensor_mul(obuf[0:64, tpair, :],
                                                 po[0:D, :], rb)
                        else:
                            tmp = ap_.tile([64, TB], F32R, tag="tmp")
                            nc.vector.tensor_mul(tmp, po[0:D, :], rb)
                            nc.gpsimd.dma_start(out=obuf[64:128, tpair, :],
                                                in_=tmp)

                def make_v(vpair, vtkc, vex, vpo2):
                    # attn@V for k-tile vtkc, shifted one step late so the PE
                    # never queues behind an in-flight exp.
                    def emit():
                        for hh in range(2):
                            nc.tensor.matmul(
                                vpo2[hh][0 : D + 1, :],
                                lhsT=v_sb[:, vtkc, vpair * 2 + hh, :],
                                rhs=vex[:, hh * 512 : (hh + 1) * 512],
                                start=(vtkc == 0), stop=(vtkc == 2 * KC - 1))
                    return emit

                pending_v = None
                pending_t = None
                for pair in range(PAIRS):
                    po0 = psB.tile([128, TB], F32, tag="po", bufs=4)
                    po1 = psB.tile([128, TB], F32, tag="po", bufs=4)
                    po2 = [po0, po1]
                    # persistent 4-bank logits tile for this pair; 2-bank
                    # halves ping-pong by k-tile parity.
                    pl = psB.tile([128, 2048], F32, tag="pl", bufs=1)
                    for tkc in range(2 * KC):
                        half = (tkc % 2) * 1024
                        ex = ap_.tile([128, 1024], BF16, tag="ex", bufs=4)
                        for hh in range(2):
                            pb = hh * 64
                            nc.tensor.matmul(
                                pl[:, half + hh * 512 : half + (hh + 1) * 512],
                                lhsT=k_sb[pb : pb + 64, pair,
                                          tkc * 128 : (tkc + 1) * 128],
                                rhs=q_sb[pb : pb + 64, pair, :],
                                start=True, stop=True)
                        nc.scalar.activation(
                            out=ex, in_=pl[:, half : half + 1024],
                            func=AF.Exp, bias=zb128[:, 0:1])
                        if pending_v is not None:
                            pending_v()
                            pending_v = None
                        if tkc == 1 and pending_t is not None:
                            emit_tail(*pending_t)
                            pending_t = None
                        pending_v = make_v(pair, tkc, ex, po2)
                    pending_t = (pair, po2)
                pending_v()
                pending_v = None
                emit_tail(*pending_t)
                pending_t = None
                # out-projection for this tq block
                for tqs in range(4):
                    osl = slice(tqs * 128, (tqs + 1) * 128)
                    osb = ap_.tile([128, 1024], F32, tag="osb", bufs=2)
                    pc = psB.tile([128, 1024], F32, tag="pl", bufs=1)
                    for nh in range(2):
                        for j in range(PAIRS):
                            nc.tensor.matmul(
                                pc[:, nh * 512 : (nh + 1) * 512],
                                lhsT=obuf[:, j, osl],
                                rhs=wo_sb[:, j, nh * 512 : (nh + 1) * 512],
                                start=(j == 0), stop=(j == PAIRS - 1))
                        if nh == 0:
                            nc.scalar.copy(out=osb[:, 0:512],
                                           in_=pc[:, 0:512])
                        else:
                            nc.vector.tensor_copy(out=osb[:, 512:1024],
                                                  in_=pc[:, 512:1024])
                    r0 = tqb * TB + tqs * 128
                    nc.sync.dma_start(out=out_d[r0 : r0 + 128, :], in_=osb)

    nc.finalize()
    return nc


def get_program():
    if "nc" not in _prog_cache:
        _prog_cache["nc"] = _build_program()
    return _prog_cache["nc"]


def _round_f32r(a):
    """Round fp32 to fp32r (E8M11: 11 mantissa bits, low 12 bits zero),
    round-to-nearest-even. Matches the PE's fp32r operand precision so the
    DMA-loaded tensors satisfy walrus's 'rounded to FP32r' requirement."""
    b = np.ascontiguousarray(a, np.float32).view(np.uint32)
    lsb = (b >> np.uint32(12)) & np.uint32(1)
    r = (b + np.uint32(0x7FF) + lsb) & np.uint32(0xFFFFF000)
    return r.view(np.float32)


def _pack_inputs(x, ln_scale, ln_bias, w_qkv, w_out, b_out):
    """Returns (in_maps for 8 cores, per-batch host bias [1024])."""
    x = np.ascontiguousarray(np.asarray(x, np.float32))
    ln_scale = np.asarray(ln_scale, np.float32)
    ln_bias = np.asarray(ln_bias, np.float32)
    w_qkv = np.asarray(w_qkv, np.float32)
    w_out = np.asarray(w_out, np.float32)
    b_out = np.asarray(b_out, np.float32)

    ws = w_qkv * ln_scale[:, None]          # fold LN scale into weights
    wq_all = ws[:, 0:1024] * (D ** -0.5)    # fold 1/sqrt(d) into q
    wk_all = ws[:, 1024:2048]
    wv_all = ws[:, 2048:3072]
    qb_all = (ln_bias @ w_qkv[:, 0:1024]) * (D ** -0.5)
    kb_all = ln_bias @ w_qkv[:, 1024:2048]
    vb_all = ln_bias @ w_qkv[:, 2048:3072]
    b_eff = (b_out + vb_all @ w_out).astype(np.float32)  # host-side bias

    in_maps = []
    for core in range(N_CORES):
        b_i, g = core // 2, core % 2
        cs = slice(g * 512, (g + 1) * 512)
        # [dim, 8 heads, 64] -> pairs of heads packed along m
        wq_g = wq_all[:, cs].reshape(DIM, PAIRS, 128)   # [dim, pair, 2*64]
        wk_g = wk_all[:, cs].reshape(DIM, PAIRS, 128)
        # -> [pair, p, kc, m] so that per-pair DMA is contiguous per partition
        wq_p = np.ascontiguousarray(
            wq_g.reshape(KC, 128, PAIRS, 128).transpose(2, 1, 0, 3))
        wk_p = np.ascontiguousarray(
            wk_g.reshape(KC, 128, PAIRS, 128).transpose(1, 0, 2, 3))
        wv_p = np.ascontiguousarray(
            wv_all[:, cs].reshape(KC, 128, 512).transpose(1, 0, 2))
        wo_p = np.ascontiguousarray(
            w_out[cs, :].reshape(PAIRS, 128, DIM).transpose(1, 0, 2))
        qb_p = np.ascontiguousarray(qb_all[cs].reshape(PAIRS, 128))
        kb_p = np.ascontiguousarray(kb_all[cs].reshape(PAIRS, 128))
        xt = np.ascontiguousarray(x[b_i].T)
        in_maps.append({
            "xt": _round_f32r(xt), "wq": _round_f32r(wq_p),
            "wk": _round_f32r(wk_p), "wv": _round_f32r(wv_p),
            "wo": _round_f32r(wo_p), "qb": qb_p, "kb": kb_p,
        })
    return in_maps, b_eff


def kernel(x, ln_scale, ln_bias, w_qkv, w_out, b_out):
    from concourse.bass_utils import run_bass_kernel_spmd

    nc = get_program()
    in_maps, b_eff = _pack_inputs(x, ln_scale, ln_bias, w_qkv, w_out, b_out)
    trace = bool(os.environ.get("ATTN_KERNEL_TRACE"))
    res = run_bass_kernel_spmd(nc, in_maps, core_ids=list(range(N_CORES)),
                               trace=trace)
    _prog_cache["last_exec_time_ns"] = res.exec_time_ns
    _prog_cache["last_result"] = res
    outs = res.results
    out = np.empty((B, N, DIM), np.float32)
    for b in range(B):
        out[b] = outs[2 * b]["out"] + outs[2 * b + 1]["out"] + b_eff
    return out


# revision 31
# speedup vs baseline: 1.2613x; 1.0952x over previous
"""Self-contained Trainium2 Bass kernel for nn_Attention_40226663694923.

Fused LayerNorm + multi-head attention + out-projection, sharded over
8 NeuronCores as (batch b in 0..3) x (head-group g in 0..1, 8 heads each).
Each core receives x[b].T plus its weight shards, computes a partial
out-projection [2048, 1024]; the host sums the two head-group partials
per batch and adds the bias.

Device-side layout is fully "transposed" (feature dim on partitions):
  - LN stats via ones-matmuls on PE (cross-partition sums), Rsqrt on ACT,
    rank-1 PE broadcast of rstd / mu*rstd rows copied to SBUF, in-place
    normalize of x^T (stats for all token blocks first, then normalize,
    so the DVE chain overlaps phase B's projection matmuls).
  - q/k projections packed two heads per matmul (M=128), v projection
    in natural layout for all 8 heads at once (N=512).
  - logitsT[tk, tq] = k^T.T-slices @ q into a persistent 4-bank PSUM
    tile (half-bank ping-pong): exp runs as two [128,1024] ACT calls per
    ck so next ck's logits overlap the current exp; attn@V via
    lhsT=[V|1] (ones column yields softmax denominators for free).
  - Tail per pair: reciprocal_approx_fast on the denominator row, PE
    rank-1 broadcast, ACT copy to SBUF, DVE multiply (no DRAM bounce).
  - out-projection from packed per-pair O tiles, full PSUM accumulation.
All matmuls run in float32r (full-rate fp32 on the PE at N>=256).
"""

import os
import sys

for _p in ("/opt/trn_rl_repo", "/root/.axon_site/_ro/trn_rl_repo"):
    if os.path.isdir(_p) and _p not in sys.path:
        sys.path.append(_p)

from contextlib import ExitStack

import numpy as np

B, N, DIM = 4, 2048, 1024
H, D = 16, 64
HPC = 8        # heads per core
PAIRS = 4      # head pairs per core
KC = 8         # 1024 dim contraction chunks of 128
NB = 4         # token blocks of 512
TB = 512
TT = 16        # token tiles of 128
LN_EPS = 1e-6
N_CORES = 8

_prog_cache = {}


def _build_program():
    import concourse.bass as bass
    import concourse.mybir as mybir
    import concourse.tile as tile
    from concourse import bacc

    F32 = mybir.dt.float32
    F32R = mybir.dt.float32r
    BF16 = mybir.dt.bfloat16
    AF = mybir.ActivationFunctionType

    nc = bacc.Bacc("TRN2", target_bir_lowering=False, debug=False)
    xt_d = nc.dram_tensor("xt", [DIM, N], F32R, kind="ExternalInput")
    wq_d = nc.dram_tensor("wq", [PAIRS, 128, KC, 128], F32R, kind="ExternalInput")
    wk_d = nc.dram_tensor("wk", [128, KC, PAIRS, 128], F32R, kind="ExternalInput")
    wv_d = nc.dram_tensor("wv", [128, KC, 512], F32R, kind="ExternalInput")
    wo_d = nc.dram_tensor("wo", [128, PAIRS, 1024], F32R, kind="ExternalInput")
    qb_d = nc.dram_tensor("qb", [PAIRS, 128], F32, kind="ExternalInput")
    kb_d = nc.dram_tensor("kb", [PAIRS, 128], F32, kind="ExternalInput")
    out_d = nc.dram_tensor("out", [N, DIM], F32, kind="ExternalOutput")

    with tile.TileContext(nc) as tc, ExitStack() as ctx:
        const_p = ctx.enter_context(tc.tile_pool(name="const", bufs=1))
        big_p = ctx.enter_context(tc.tile_pool(name="big", bufs=1))

        onesF = const_p.tile([128, 128], F32)
        nc.vector.memset(onesF, 1.0)
        ones_col = const_p.tile([128, 1], F32R)
        nc.vector.tensor_copy(out=ones_col, in_=onesF[:, 0:1])
        ones_row = const_p.tile([1, 128], F32R)
        nc.vector.tensor_copy(out=ones_row, in_=onesF[0:1, :])
        ones_row_bf = const_p.tile([1, 128], BF16)
        nc.vector.tensor_copy(out=ones_row_bf, in_=onesF[0:1, :])
        eps1 = const_p.tile([1, 1], F32)
        nc.vector.memset(eps1, LN_EPS)
        zb128 = const_p.tile([128, 1], F32)
        nc.vector.memset(zb128, 0.0)
        qb_sb = const_p.tile([128, PAIRS], F32)
        kb_sb = const_p.tile([128, PAIRS], F32)
        for pair in range(PAIRS):
            nc.gpsimd.dma_start(
                out=qb_sb[:, pair : pair + 1],
                in_=qb_d[pair, :].rearrange("(p one) -> p one", one=1),
            )
            nc.gpsimd.dma_start(
                out=kb_sb[:, pair : pair + 1],
                in_=kb_d[pair, :].rearrange("(p one) -> p one", one=1),
            )

        # persistent big tensors
        xt_sb = big_p.tile([128, KC, N], F32R)      # becomes z (normalized) in place
        k_sb = big_p.tile([128, PAIRS, N], BF16)    # kT, two heads packed per pair
        v_sb = big_p.tile([128, TT, HPC, D + 1], BF16)  # V natural + ones column
        wo_sb = big_p.tile([128, PAIRS, 1024], F32R)

        # xt: per-(kc, tb) pieces so phase A's stats for tb=0 can start after
        # only 8 small DMAs (tb-major issue order). sync queue.
        for tb in range(NB):
            for kc in range(KC):
                nc.sync.dma_start(
                    out=xt_sb[:, kc, tb * TB : (tb + 1) * TB],
                    in_=xt_d[kc * 128 : (kc + 1) * 128, tb * TB : (tb + 1) * TB],
                )
        nc.sync.dma_start(out=wo_sb, in_=wo_d[:, :, :])
        nc.vector.tensor_copy(
            out=v_sb[:, :, :, D : D + 1],
            in_=onesF.rearrange("p (a b c) -> p a b c", a=TT, b=HPC),
        )
        # k/v weights on the gpsimd queue so they don't sit behind xt; the
        # wkv pool closes after phase B so its SBUF is reused by phase C.
        wkv_ctx = tc.tile_pool(name="wkv", bufs=1)
        wkv_p = wkv_ctx.__enter__()
        wv_sb = wkv_p.tile([128, KC, 512], F32R, tag="wv")
        nc.gpsimd.dma_start(out=wv_sb, in_=wv_d[:, :, :])
        wk_sb = wkv_p.tile([128, KC, PAIRS, 128], F32R, tag="wk")
        nc.gpsimd.dma_start(out=wk_sb, in_=wk_d[:, :, :, :])

        # ---------------- Phase A: LN stats then in-place normalize ----------
        with tc.tile_pool(name="psA", bufs=2, space="PSUM") as psA, \
             tc.tile_pool(name="sqp", bufs=3) as sqp, \
             tc.tile_pool(name="rows", bufs=1) as rows:
            rb_tiles = []
            for tb in range(NB):
                ts_ = slice(tb * TB, (tb + 1) * TB)
                s1 = psA.tile([1, TB], F32, tag="s1", bufs=1)
                s2 = psA.tile([1, TB], F32, tag="s2", bufs=1)
                for kc in range(KC):
                    sq = sqp.tile([128, TB], F32R, tag="sq")
                    nc.scalar.activation(out=sq, in_=xt_sb[:, kc, ts_].bitcast(F32),
                                         func=AF.Square, bias=zb128[:, 0:1])
                    nc.tensor.matmul(s1, lhsT=ones_col, rhs=xt_sb[:, kc, ts_],
                                     start=(kc == 0), stop=(kc == KC - 1))
                    nc.tensor.matmul(s2, lhsT=ones_col, rhs=sq,
                                     start=(kc == 0), stop=(kc == KC - 1))
                mu = rows.tile([1, TB], F32, tag="mu")
                nc.vector.tensor_scalar_mul(mu, s1, 1.0 / DIM)
                ex2 = rows.tile([1, TB], F32, tag="ex2")
                nc.vector.tensor_scalar_mul(ex2, s2, 1.0 / DIM)
                var_r = rows.tile([1, TB], F32, tag="var")
                nc.vector.tensor_mul(var_r, mu, mu)
                nc.vector.tensor_sub(var_r, ex2, var_r)
                sd = rows.tile([1, TB], F32, tag="sd")
                nc.scalar.activation(out=sd, in_=var_r, func=AF.Sqrt,
                                     bias=eps1[0:1, 0:1])
                rstd_r = rows.tile([1, TB], F32, tag="rstd_r")
                nc.vector.reciprocal(out=rstd_r, in_=sd)
                murstd_r = rows.tile([1, TB], F32R, tag="murstd")
                nc.vector.tensor_mul(murstd_r, mu, rstd_r)
                rstd_rr = rows.tile([1, TB], F32R, tag="rstd_rr")
                nc.vector.tensor_copy(out=rstd_rr, in_=rstd_r)
                rb1 = psA.tile([128, TB], F32, tag="rb1", bufs=3)
                nc.tensor.matmul(rb1, lhsT=ones_row, rhs=rstd_rr,
                                 start=True, stop=True)
                rb2 = psA.tile([128, TB], F32, tag="rb2", bufs=3)
                nc.tensor.matmul(rb2, lhsT=ones_row, rhs=murstd_r,
                                 start=True, stop=True)
                rb_tiles.append((rb1, rb2))
            for tb in range(NB):
                ts_ = slice(tb * TB, (tb + 1) * TB)
                rb1, rb2 = rb_tiles[tb]
                for kc in range(KC):
                    nc.vector.tensor_mul(xt_sb[:, kc, ts_],
                                         xt_sb[:, kc, ts_].bitcast(F32), rb1)
                    nc.vector.tensor_sub(xt_sb[:, kc, ts_],
                                         xt_sb[:, kc, ts_].bitcast(F32), rb2)

        # ---------------- Phase B: k and v projections ------------------------
        psB = ctx.enter_context(tc.tile_pool(name="psB", bufs=2, space="PSUM"))
        for tb in range(NB):
            ts_ = slice(tb * TB, (tb + 1) * TB)
            for pair in range(PAIRS):
                pk = psB.tile([128, TB], F32, tag="po", bufs=4)
                for kc in range(KC):
                    nc.tensor.matmul(pk, lhsT=wk_sb[:, kc, pair, :],
                                     rhs=xt_sb[:, kc, ts_],
                                     start=(kc == 0), stop=(kc == KC - 1))
                nc.vector.tensor_scalar_add(out=k_sb[:, pair, ts_], in0=pk,
                                            scalar1=kb_sb[:, pair : pair + 1])
            for tt in range(tb * 4, tb * 4 + 4):
                tts = slice(tt * 128, (tt + 1) * 128)
                pv = psB.tile([128, 512], F32, tag="po", bufs=4)
                for kc in range(KC):
                    nc.tensor.matmul(pv, lhsT=xt_sb[:, kc, tts],
                                     rhs=wv_sb[:, kc, :],
                                     start=(kc == 0), stop=(kc == KC - 1))
                nc.vector.tensor_copy(
                    out=v_sb[:, tt, :, 0:D],
                    in_=pv.rearrange("p (h d) -> p h d", h=HPC),
                )
        wkv_ctx.__exit__(None, None, None)

        # ---------------- Phase C: attention + out-projection -----------------
        with tc.tile_pool(name="attn", bufs=2) as ap_, \
             tc.tile_pool(name="rows2", bufs=2) as rows2, \
             tc.tile_pool(name="drbounce", bufs=4, space="DRAM") as dram_p:
            phase_state = {}
            for tqb in range(NB):
                tqs_ = slice(tqb * TB, (tqb + 1) * TB)
                obuf = ap_.tile([128, PAIRS, TB], F32R, tag="ob", bufs=1)
                # --- all 4 q projections upfront, into one pl-tag slot ------
                pq4 = psB.tile([128, 2048], F32, tag="pl", bufs=1)
                q_sb = ap_.tile([128, PAIRS, TB], BF16, tag="q", bufs=1)
                if phase_state.get("pending_out") is not None:
                    _po_emit = phase_state.pop("pending_out")
                else:
                    _po_emit = None
                for pair in range(PAIRS):
                    wq_sb = ap_.tile([128, KC, 128], F32R, tag="wq")
                    nc.sync.dma_start(out=wq_sb, in_=wq_d[pair])
                    for kc in range(KC):
                        nc.tensor.matmul(pq4[:, pair * TB : (pair + 1) * TB],
                                         lhsT=wq_sb[:, kc, :],
                                         rhs=xt_sb[:, kc, tqs_],
                                         start=(kc == 0), stop=(kc == KC - 1))
                    nc.vector.tensor_scalar_add(
                        out=q_sb[:, pair, :],
                        in0=pq4[:, pair * TB : (pair + 1) * TB],
                        scalar1=qb_sb[:, pair : pair + 1])
                if pair == PAIRS - 1 and _po_emit is not None:
                    _po_emit()
                    _po_emit = None
                def emit_tail(tpair, tpo2):
                    # Softmax-normalize tail with NO PE work: reciprocal of
                    # the ones-row on DVE, row-broadcast via a DRAM-bounce
                    # DMA pair (gpsimd queue is otherwise idle), DVE multiply.
                    import concourse.bass as _b
                    for hh in range(2):
                        po = tpo2[hh]
                        rrow = rows2.tile([1, TB], F32, tag="rr")
                        nc.vector.reciprocal(out=rrow, in_=po[D : D + 1, :])
                        dr = dram_p.tile([1, TB], F32, tag="dr")
                        nc.gpsimd.dma_start(out=dr, in_=rrow)
                        rb = ap_.tile([64, TB], F32, tag="rb", bufs=2)
                        bc = _b.AP(tensor=dr.tensor, offset=dr.offset,
                                   ap=[[0, 64]] + [list(p) for p in dr[0, :].ap])
                        nc.gpsimd.dma_start(out=rb, in_=bc)
                        if hh == 0:
                            nc.vector.tensor_mul(obuf[0:64, tpair, :],
                                                 po[0:D, :], rb)
                        else:
                            tmp = ap_.tile([64, TB], F32R, tag="tmp")
                            nc.vector.tensor_mul(tmp, po[0:D, :], rb)
                            nc.gpsimd.dma_start(out=obuf[64:128, tpair, :],
                                                in_=tmp)

                def make_v(vpair, vtkc, vex, vpo2):
                    # attn@V for k-tile vtkc, shifted one step late so the PE
                    # never queues behind an in-flight exp.
                    def emit():
                        for hh in range(2):
                            nc.tensor.matmul(
                                vpo2[hh][0 : D + 1, :],
                                lhsT=v_sb[:, vtkc, vpair * 2 + hh, :],
                                rhs=vex[:, hh * 512 : (hh + 1) * 512],
                                start=(vtkc == 0), stop=(vtkc == 2 * KC - 1))
                    return emit

                pending_v = None
                pending_t = None
                for pair in range(PAIRS):
                    po0 = psB.tile([128, TB], F32, tag="po", bufs=4)
                    po1 = psB.tile([128, TB], F32, tag="po", bufs=4)
                    po2 = [po0, po1]
                    # persistent 4-bank logits tile for this pair; 2-bank
                    # halves ping-pong by k-tile parity.
                    pl = psB.tile([128, 2048], F32, tag="pl", bufs=1)
                    for tkc in range(2 * KC):
                        half = (tkc % 2) * 1024
                        ex = ap_.tile([128, 1024], BF16, tag="ex", bufs=4)
                        for hh in range(2):
                            pb = hh * 64
                            nc.tensor.matmul(
                                pl[:, half + hh * 512 : half + (hh + 1) * 512],
                                lhsT=k_sb[pb : pb + 64, pair,
                                          tkc * 128 : (tkc + 1) * 128],
                                rhs=q_sb[pb : pb + 64, pair, :],
                                start=True, stop=True)
                        nc.scalar.activation(
                            out=ex, in_=pl[:, half : half + 1024],
                            func=AF.Exp, bias=zb128[:, 0:1])
                        if pending_v is not None:
                            pending_v()
                            pending_v = None
                        if tkc == 1 and pending_t is not None:
                            emit_tail(*pending_t)
                            pending_t = None
                        pending_v = make_v(pair, tkc, ex, po2)
                    pending_t = (pair, po2)
                pending_v()
                pending_v = None
                emit_tail(*pending_t)
                pending_t = None
                def make_outproj(otqb, oobuf):
                    def emit():
                        for tqs in range(4):
                            osl = slice(tqs * 128, (tqs + 1) * 128)
                            osb = ap_.tile([128, 1024], F32, tag="osb", bufs=2)
                            pc0 = psB.tile([128, TB], F32, tag="po", bufs=4)
                            pc1 = psB.tile([128, TB], F32, tag="po", bufs=4)
                            for nh, pc in ((0, pc0), (1, pc1)):
                                for j in range(PAIRS):
                                    nc.tensor.matmul(
                                        pc, lhsT=oobuf[:, j, osl],
                                        rhs=wo_sb[:, j, nh * 512 : (nh + 1) * 512],
                                        start=(j == 0), stop=(j == PAIRS - 1))
                                if nh == 0:
                                    nc.scalar.copy(out=osb[:, 0:512], in_=pc)
                                else:
                                    nc.vector.tensor_copy(out=osb[:, 512:1024],
                                                          in_=pc)
                            r0 = otqb * TB + tqs * 128
                            nc.sync.dma_start(out=out_d[r0 : r0 + 128, :],
                                              in_=osb)
                    return emit

                pending_out_new = make_outproj(tqb, obuf)
                if tqb == NB - 1:
                    pending_out_new()
                else:
                    phase_state["pending_out"] = pending_out_new
    nc.finalize()
    return nc


def get_program():
    if "nc" not in _prog_cache:
        _prog_cache["nc"] = _build_program()
    return _prog_cache["nc"]


def _round_f32r(a):
    """Round fp32 to fp32r (E8M11: 11 mantissa bits, low 12 bits zero),
    round-to-nearest-even. Matches the PE's fp32r operand precision so the
    DMA-loaded tensors satisfy walrus's 'rounded to FP32r' requirement."""
    b = np.ascontiguousarray(a, np.float32).view(np.uint32)
    lsb = (b >> np.uint32(12)) & np.uint32(1)
    r = (b + np.uint32(0x7FF) + lsb) & np.uint32(0xFFFFF000)
    return r.view(np.float32)


def _pack_inputs(x, ln_scale, ln_bias, w_qkv, w_out, b_out):
    """Returns (in_maps for 8 cores, per-batch host bias [1024])."""
    x = np.ascontiguousarray(np.asarray(x, np.float32))
    ln_scale = np.asarray(ln_scale, np.float32)
    ln_bias = np.asarray(ln_bias, np.float32)
    w_qkv = np.asarray(w_qkv, np.float32)
    w_out = np.asarray(w_out, np.float32)
    b_out = np.asarray(b_out, np.float32)

    ws = w_qkv * ln_scale[:, None]          # fold LN scale into weights
    wq_all = ws[:, 0:1024] * (D ** -0.5)    # fold 1/sqrt(d) into q
    wk_all = ws[:, 1024:2048]
    wv_all = ws[:, 2048:3072]
    qb_all = (ln_bias @ w_qkv[:, 0:1024]) * (D ** -0.5)
    kb_all = ln_bias @ w_qkv[:, 1024:2048]
    vb_all = ln_bias @ w_qkv[:, 2048:3072]
    b_eff = (b_out + vb_all @ w_out).astype(np.float32)  # host-side bias

    in_maps = []
    for core in range(N_CORES):
        b_i, g = core // 2, core % 2
        cs = slice(g * 512, (g + 1) * 512)
        # [dim, 8 heads, 64] -> pairs of heads packed along m
        wq_g = wq_all[:, cs].reshape(DIM, PAIRS, 128)   # [dim, pair, 2*64]
        wk_g = wk_all[:, cs].reshape(DIM, PAIRS, 128)
        # -> [pair, p, kc, m] so that per-pair DMA is contiguous per partition
        wq_p = np.ascontiguousarray(
            wq_g.reshape(KC, 128, PAIRS, 128).transpose(2, 1, 0, 3))
        wk_p = np.ascontiguousarray(
            wk_g.reshape(KC, 128, PAIRS, 128).transpose(1, 0, 2, 3))
        wv_p = np.ascontiguousarray(
            wv_all[:, cs].reshape(KC, 128, 512).transpose(1, 0, 2))
        wo_p = np.ascontiguousarray(
            w_out[cs, :].reshape(PAIRS, 128, DIM).transpose(1, 0, 2))
        qb_p = np.ascontiguousarray(qb_all[cs].reshape(PAIRS, 128))
        kb_p = np.ascontiguousarray(kb_all[cs].reshape(PAIRS, 128))
        xt = np.ascontiguousarray(x[b_i].T)
        in_maps.append({
            "xt": _round_f32r(xt), "wq": _round_f32r(wq_p),
            "wk": _round_f32r(wk_p), "wv": _round_f32r(wv_p),
            "wo": _round_f32r(wo_p), "qb": qb_p, "kb": kb_p,
        })
    return in_maps, b_eff


def kernel(x, ln_scale, ln_bias, w_qkv, w_out, b_out):
    from concourse.bass_utils import run_bass_kernel_spmd

    nc = get_program()
    in_maps, b_eff = _pack_inputs(x, ln_scale, ln_bias, w_qkv, w_out, b_out)
    trace = bool(os.environ.get("ATTN_KERNEL_TRACE"))
    res = run_bass_kernel_spmd(nc, in_maps, core_ids=list(range(N_CORES)),
                               trace=trace)
    _prog_cache["last_exec_time_ns"] = res.exec_time_ns
    _prog_cache["last_result"] = res
    outs = res.results
    out = np.empty((B, N, DIM), np.float32)
    for b in range(B):
        out[b] = outs[2 * b]["out"] + outs[2 * b + 1]["out"] + b_eff
    return out
